# revision 4
# baseline (speedup 1.0000x reference)
"""Trainium2 Bass kernel for nn_ExplodedLogit (topk_masking).

Reference computation (x (512,256) f32, W (1,256) f32, b (1,) f32):
    scores = x @ W.T + b                                  (512, 1)
    idx    = argmax(scores)
    mask   = ones(512) with log(1e-46) at idx
    block  = scores * mask[None, :]                       (512, 512)
    out    = concat([scores, tile(block, (1, 512))], 1)   (512, 262145)

Sharding: the 512 identical block repetitions are split across 8
NeuronCores, 64 reps each -> per-core "rep" output (512, 32768) = 64 MB
(memory-bound: the kernel is an HBM-write problem). Every core runs the
identical program: scores/argmax/mask are recomputed redundantly (tiny),
and the per-core slice is materialized with fan-out DMAs that read a
small SBUF block through a step-0 (broadcast) access-pattern dim.

Row layout trick: row r = 4p + t (p = partition 0..127, t = 0..3) so the
(512,) scores vector lands contiguously in DRAM from a [128,4] SBUF tile,
enabling the cross-partition broadcast needed for the mask without any
transpose.
"""

import math

import numpy as np

import concourse.bacc as bacc
import concourse.mybir as mybir
import concourse.tile as tile
from concourse.bass_utils import run_bass_kernel_spmd

F32 = mybir.dt.float32
MASK_VAL = float(np.float32(math.log(1e-46)))  # ~ -105.9189

T = 512        # tracks (rows)
F = 256        # features
P = 128        # SBUF partitions
TPP = T // P   # 4 rows per partition (r = 4p + t)
NREP = 512     # total block repetitions in the full output
NCORES = 8
RPC = NREP // NCORES   # 64 reps per core
R = 8                  # reps materialized in SBUF
G = RPC // R           # step-0 groups per fan-out DMA


def _build():
    nc = bacc.Bacc("TRN2", target_bir_lowering=False, debug=False)
    x = nc.dram_tensor("x", [T, F], F32, kind="ExternalInput")
    W = nc.dram_tensor("W", [1, F], F32, kind="ExternalInput")
    b = nc.dram_tensor("b", [1, 1], F32, kind="ExternalInput")
    rep_out = nc.dram_tensor("rep", [T, RPC * T], F32, kind="ExternalOutput")
    scores_out = nc.dram_tensor("scores", [T, 1], F32, kind="ExternalOutput")

    with tile.TileContext(nc) as tc:
        with (
            tc.tile_pool(name="dram", bufs=1, space="DRAM") as dram_pool,
            tc.tile_pool(name="sbuf", bufs=1) as sbuf_pool,
        ):
            _emit(nc, x[:], W[:], b[:], rep_out[:], scores_out[:],
                  dram_pool, sbuf_pool)
    nc.compile()
    return nc


def _emit(nc, x, W, b, rep_out, scores_out, dram_pool, sbuf_pool):
    x_sb = sbuf_pool.tile([P, TPP * F], F32)     # x[4p+t, f] at [p, t*F+f]
    w_sb = sbuf_pool.tile([P, TPP * F], F32)     # W replicated 4x in free dim
    b_sb = sbuf_pool.tile([P, 1], F32)
    tmp_sb = sbuf_pool.tile([P, TPP * F], F32)
    sc_sb = sbuf_pool.tile([P, TPP], F32)        # scores: s[4p+t] at [p,t]
    sbc_sb = sbuf_pool.tile([P, T], F32)         # scores bcast: s[c] at [p,c]
    m8_sb = sbuf_pool.tile([P, 8], F32)
    mask_sb = sbuf_pool.tile([P, T], F32)
    rep_sb = sbuf_pool.tile([P, TPP * R * T], F32)

    scratch = dram_pool.tile([T], F32)           # scores in row order

    # ---- loads ----
    nc.sync.dma_start(x_sb[:], x.rearrange("(p t) f -> p (t f)", p=P))
    nc.sync.dma_start(
        w_sb[:], W.unsqueeze(1).broadcast_to((P, TPP, F))
    )
    nc.sync.dma_start(b_sb[:], b.broadcast_to((P, 1)))

    # ---- scores: s[4p+t] = b + sum_f x[4p+t,f] * W[f] ----
    # (tensor_tensor_reduce would fuse this, but it hard-crashes the
    #  device here — NRT_EXEC_UNIT_UNRECOVERABLE — so mul + reduce.)
    nc.vector.tensor_mul(tmp_sb[:], x_sb[:], w_sb[:])
    nc.vector.reduce_sum(
        sc_sb[:], tmp_sb[:].rearrange("p (t f) -> p t f", t=TPP),
        axis=mybir.AxisListType.X,
    )
    nc.vector.tensor_scalar_add(sc_sb[:], sc_sb[:], b_sb[:, 0:1])

    # ---- scores -> DRAM (row order) + external scores output ----
    nc.sync.dma_start(scratch[:].rearrange("(p t) -> p t", p=P), sc_sb[:])
    nc.sync.dma_start(
        scores_out.rearrange("(p t) one -> p (t one)", p=P), sc_sb[:]
    )

    # ---- broadcast scores to all partitions, build mask ----
    nc.sync.dma_start(sbc_sb[:], scratch[:].unsqueeze(0).broadcast_to((P, T)))
    nc.vector.max(m8_sb[:], sbc_sb[:])
    # ind = (s == max); mask = ind * (MASK_VAL-1) + 1  (exact in f32 here)
    nc.vector.tensor_scalar(
        mask_sb[:], sbc_sb[:], m8_sb[:, 0:1], None, mybir.AluOpType.is_equal
    )
    nc.vector.tensor_scalar(
        mask_sb[:], mask_sb[:], MASK_VAL - 1.0, 1.0,
        mybir.AluOpType.mult, mybir.AluOpType.add,
    )

    # ---- fill rep_sb: R copies of each row's block slice ----
    # rep_sb[p, (t*R+r)*T + c] = sc[p,t] * mask[c]
    for t in range(TPP):
        out_ap = rep_sb[:, t * R * T:(t + 1) * R * T].rearrange(
            "p (r c) -> p r c", c=T
        )
        in_ap = mask_sb[:].unsqueeze(1).broadcast_to((P, R, T))
        nc.vector.tensor_scalar(
            out_ap, in_ap, sc_sb[:, t:t + 1], None, mybir.AluOpType.mult
        )

    # ---- fan-out DMAs: write each t-slot G times via a step-0 src dim ----
    out_v = rep_out.rearrange("(p t) (g q) -> t p g q", p=P, q=R * T)
    for t in range(TPP):
        src = (
            rep_sb[:, t * R * T:(t + 1) * R * T]
            .unsqueeze(1)
            .broadcast_to((P, G, R * T))
        )
        nc.sync.dma_start(out_v[t], src)


_NC_CACHE = None


def _get_nc():
    global _NC_CACHE
    if _NC_CACHE is None:
        _NC_CACHE = _build()
    return _NC_CACHE


def _run(x, W, b, **run_kwargs):
    nc = _get_nc()
    in_map = {
        "x": np.ascontiguousarray(np.asarray(x, dtype=np.float32)),
        "W": np.ascontiguousarray(np.asarray(W, dtype=np.float32)).reshape(1, F),
        "b": np.ascontiguousarray(np.asarray(b, dtype=np.float32)).reshape(1, 1),
    }
    return run_bass_kernel_spmd(
        nc,
        [dict(in_map) for _ in range(NCORES)],
        core_ids=list(range(NCORES)),
        **run_kwargs,
    )


def kernel(x, W, b):
    res = _run(x, W, b)
    outs = res.results
    full = np.empty((T, 1 + NREP * T), dtype=np.float32)
    full[:, 0:1] = outs[0]["scores"]
    for c in range(NCORES):
        full[:, 1 + c * RPC * T: 1 + (c + 1) * RPC * T] = outs[c]["rep"]
    return full


# revision 6
# speedup vs baseline: 1.3759x; 1.3759x over previous
"""Trainium2 Bass kernel for nn_ExplodedLogit (topk_masking).

Reference computation (x (512,256) f32, W (1,256) f32, b (1,) f32):
    scores = x @ W.T + b                                  (512, 1)
    idx    = argmax(scores)
    mask   = ones(512) with log(1e-46) at idx
    block  = scores * mask[None, :]                       (512, 512)
    out    = concat([scores, tile(block, (1, 512))], 1)   (512, 262145)

Sharding: the 512 identical block repetitions are split across 8
NeuronCores, 64 reps each -> per-core "rep" output (512, 32768) = 64 MB
(memory-bound: the kernel is an HBM-write problem). Every core runs the
identical program: scores/argmax/mask are recomputed redundantly (tiny),
and the per-core slice is materialized with fan-out DMAs that read a
small SBUF block through a step-0 (broadcast) access-pattern dim.

Row layout trick: row r = 4p + t (p = partition 0..127, t = 0..3) so the
(512,) scores vector lands contiguously in DRAM from a [128,4] SBUF tile,
enabling the cross-partition broadcast needed for the mask without any
transpose.
"""

import math

import numpy as np

import concourse.bacc as bacc
import concourse.mybir as mybir
import concourse.tile as tile
from concourse.bass_utils import run_bass_kernel_spmd

F32 = mybir.dt.float32
MASK_VAL = float(np.float32(math.log(1e-46)))  # ~ -105.9189

T = 512        # tracks (rows)
F = 256        # features
P = 128        # SBUF partitions
TPP = T // P   # 4 rows per partition (r = 4p + t)
NREP = 512     # total block repetitions in the full output
NCORES = 8
RPC = NREP // NCORES   # 64 reps per core
R = 8                  # reps materialized in SBUF
G = RPC // R           # step-0 groups per fan-out DMA


def _build():
    nc = bacc.Bacc("TRN2", target_bir_lowering=False, debug=False)
    x = nc.dram_tensor("x", [T, F], F32, kind="ExternalInput")
    W = nc.dram_tensor("W", [1, F], F32, kind="ExternalInput")
    b = nc.dram_tensor("b", [1, 1], F32, kind="ExternalInput")
    rep_out = nc.dram_tensor("rep", [T, RPC * T], F32, kind="ExternalOutput")
    scores_out = nc.dram_tensor("scores", [T, 1], F32, kind="ExternalOutput")

    with tile.TileContext(nc) as tc:
        with (
            tc.tile_pool(name="dram", bufs=1, space="DRAM") as dram_pool,
            tc.tile_pool(name="sbuf", bufs=1) as sbuf_pool,
        ):
            _emit(nc, x[:], W[:], b[:], rep_out[:], scores_out[:],
                  dram_pool, sbuf_pool)
    nc.compile()
    return nc


def _emit(nc, x, W, b, rep_out, scores_out, dram_pool, sbuf_pool):
    x_sb = sbuf_pool.tile([P, TPP * F], F32)     # x[4p+t, f] at [p, t*F+f]
    w_sb = sbuf_pool.tile([P, F], F32)
    b_sb = sbuf_pool.tile([P, 1], F32)
    tmp_sb = sbuf_pool.tile([P, TPP * F], F32)
    sc_sb = sbuf_pool.tile([P, TPP], F32)        # scores: s[4p+t] at [p,t]
    sbc_sb = sbuf_pool.tile([P, T], F32)         # scores bcast: s[c] at [p,c]
    m8_sb = sbuf_pool.tile([P, 8], F32)
    mask_sb = sbuf_pool.tile([P, T], F32)
    rep_sb = sbuf_pool.tile([P, TPP * R * T], F32)

    scratch = dram_pool.tile([T], F32)           # scores in row order

    # ---- loads ----
    nc.sync.dma_start(x_sb[:], x.rearrange("(p t) f -> p (t f)", p=P))
    nc.sync.dma_start(w_sb[:], W.broadcast_to((P, F)))
    nc.sync.dma_start(b_sb[:], b.broadcast_to((P, 1)))

    # ---- scores: s[4p+t] = b + sum_f x[4p+t,f] * W[f] ----
    # (tensor_tensor_reduce would fuse this, but it hard-crashes the
    #  device here — NRT_EXEC_UNIT_UNRECOVERABLE — so mul + reduce.)
    nc.vector.tensor_mul(
        tmp_sb[:].rearrange("p (t f) -> p t f", t=TPP),
        x_sb[:].rearrange("p (t f) -> p t f", t=TPP),
        w_sb[:].unsqueeze(1).broadcast_to((P, TPP, F)),
    )
    nc.vector.reduce_sum(
        sc_sb[:], tmp_sb[:].rearrange("p (t f) -> p t f", t=TPP),
        axis=mybir.AxisListType.X,
    )
    nc.vector.tensor_scalar_add(sc_sb[:], sc_sb[:], b_sb[:, 0:1])

    # ---- scores -> DRAM (row order) + external scores output ----
    nc.sync.dma_start(scratch[:].rearrange("(p t) -> p t", p=P), sc_sb[:])
    nc.sync.dma_start(
        scores_out.rearrange("(p t) one -> p (t one)", p=P), sc_sb[:]
    )

    # ---- broadcast scores to all partitions, build mask ----
    nc.sync.dma_start(sbc_sb[:], scratch[:].unsqueeze(0).broadcast_to((P, T)))
    nc.vector.max(m8_sb[:], sbc_sb[:])
    # ind = (s == max); mask = ind * (MASK_VAL-1) + 1  (exact in f32 here)
    nc.vector.tensor_scalar(
        mask_sb[:], sbc_sb[:], m8_sb[:, 0:1], None, mybir.AluOpType.is_equal
    )
    nc.vector.tensor_scalar(
        mask_sb[:], mask_sb[:], MASK_VAL - 1.0, 1.0,
        mybir.AluOpType.mult, mybir.AluOpType.add,
    )

    # ---- fill rep_sb: R copies of each row's block slice ----
    # rep_sb[p, (t*R+r)*T + c] = sc[p,t] * mask[c]
    for t in range(TPP):
        out_ap = rep_sb[:, t * R * T:(t + 1) * R * T].rearrange(
            "p (r c) -> p r c", c=T
        )
        in_ap = mask_sb[:].unsqueeze(1).broadcast_to((P, R, T))
        nc.vector.tensor_scalar(
            out_ap, in_ap, sc_sb[:, t:t + 1], None, mybir.AluOpType.mult
        )

    # ---- fan-out DMAs: write each t-slot G times via a step-0 src dim ----
    out_v = rep_out.rearrange("(p t) (g q) -> t p g q", p=P, q=R * T)
    for t in range(TPP):
        src = (
            rep_sb[:, t * R * T:(t + 1) * R * T]
            .unsqueeze(1)
            .broadcast_to((P, G, R * T))
        )
        nc.sync.dma_start(out_v[t], src)


_NC_CACHE = None


def _get_nc():
    global _NC_CACHE
    if _NC_CACHE is None:
        _NC_CACHE = _build()
    return _NC_CACHE


def _run(x, W, b, **run_kwargs):
    nc = _get_nc()
    in_map = {
        "x": np.ascontiguousarray(np.asarray(x, dtype=np.float32)),
        "W": np.ascontiguousarray(np.asarray(W, dtype=np.float32)).reshape(1, F),
        "b": np.ascontiguousarray(np.asarray(b, dtype=np.float32)).reshape(1, 1),
    }
    return run_bass_kernel_spmd(
        nc,
        [dict(in_map) for _ in range(NCORES)],
        core_ids=list(range(NCORES)),
        **run_kwargs,
    )


def kernel(x, W, b):
    res = _run(x, W, b)
    outs = res.results
    full = np.empty((T, 1 + NREP * T), dtype=np.float32)
    full[:, 0:1] = outs[0]["scores"]
    for c in range(NCORES):
        full[:, 1 + c * RPC * T: 1 + (c + 1) * RPC * T] = outs[c]["rep"]
    return full


# revision 11
# speedup vs baseline: 1.4332x; 1.0417x over previous
"""Trainium2 Bass kernel for nn_ExplodedLogit (topk_masking).

Reference computation (x (512,256) f32, W (1,256) f32, b (1,) f32):
    scores = x @ W.T + b                                  (512, 1)
    idx    = argmax(scores)
    mask   = ones(512) with log(1e-46) at idx
    block  = scores * mask[None, :]                       (512, 512)
    out    = concat([scores, tile(block, (1, 512))], 1)   (512, 262145)

Sharding: the 512 identical block repetitions are split across 8
NeuronCores, 64 reps each -> per-core "rep" output (512, 32768) = 64 MB
(memory-bound: this is an HBM-write problem). Every core runs the
identical program: scores/argmax/mask are recomputed redundantly (tiny),
and the per-core slice is materialized with fan-out DMAs that read a
small SBUF block through a step-0 (broadcast) access-pattern dim.

Row layout: r = 128t + p (p = partition 0..127, t = 0..3). The
cross-partition broadcast of the 512 scores (needed to build the mask
along the free dim) runs entirely on-chip: PE transposes scores
[128,4] -> [4,128], then four selector matmuls broadcast each
128-score chunk to all partitions in PSUM — no DRAM round-trip.
"""

import math

import numpy as np

import concourse.bacc as bacc
import concourse.mybir as mybir
import concourse.tile as tile
from concourse.bass_utils import run_bass_kernel_spmd

F32 = mybir.dt.float32
MASK_VAL = float(np.float32(math.log(1e-46)))  # ~ -105.9189

T = 512        # tracks (rows)
F = 256        # features
P = 128        # SBUF partitions
TPP = T // P   # 4 rows per partition (r = 128t + p)
NREP = 512     # total block repetitions in the full output
NCORES = 8
RPC = NREP // NCORES   # 64 reps per core
R = 8                  # reps materialized in SBUF
G = RPC // R           # step-0 groups per fan-out DMA


def _build():
    nc = bacc.Bacc("TRN2", target_bir_lowering=False, debug=False)
    x = nc.dram_tensor("x", [T, F], F32, kind="ExternalInput")
    W = nc.dram_tensor("W", [1, F], F32, kind="ExternalInput")
    b = nc.dram_tensor("b", [1, 1], F32, kind="ExternalInput")
    rep_out = nc.dram_tensor("rep", [T, RPC * T], F32, kind="ExternalOutput")
    scores_out = nc.dram_tensor("scores", [T, 1], F32, kind="ExternalOutput")

    with tile.TileContext(nc) as tc:
        with (
            tc.tile_pool(name="sbuf", bufs=1) as sbuf_pool,
            tc.tile_pool(name="psum", bufs=1, space="PSUM") as psum_pool,
        ):
            _emit(nc, x[:], W[:], b[:], rep_out[:], scores_out[:],
                  sbuf_pool, psum_pool)
    nc.compile()
    return nc


def _emit(nc, x, W, b, rep_out, scores_out, sbuf_pool, psum_pool):
    x_sb = sbuf_pool.tile([P, TPP * F], F32)     # x[128t+p, f] at [p, t*F+f]
    w_sb = sbuf_pool.tile([P, F], F32)
    b_sb = sbuf_pool.tile([P, 1], F32)
    tmp_sb = sbuf_pool.tile([P, TPP * F], F32)
    sc_sb = sbuf_pool.tile([P, TPP], F32)        # scores: s[128t+p] at [p,t]
    ones_sb = sbuf_pool.tile([P, P], F32)
    id_sb = sbuf_pool.tile([P, P], F32)          # 128x128 identity
    onesk_sb = sbuf_pool.tile([TPP, TPP * P], F32)
    sel_sb = sbuf_pool.tile([TPP, TPP * P], F32)  # selector one-hot rows
    s4_sb = sbuf_pool.tile([TPP, P], F32)        # scores, free-dim chunks
    sbc_sb = sbuf_pool.tile([P, T], F32)         # scores bcast: s[c] at [p,c]
    m8_sb = sbuf_pool.tile([P, 8], F32)
    mask_sb = sbuf_pool.tile([P, T], F32)
    rep_sb = sbuf_pool.tile([P, TPP * R * T], F32)

    sT_ps = psum_pool.tile([TPP, P], F32)
    sbc_ps = psum_pool.tile([P, T], F32)

    # ---- constants (overlap with the x load) ----
    nc.vector.memset(ones_sb[:], 1.0)
    # identity: keep ones where (col - row) == 0
    nc.gpsimd.affine_select(
        id_sb[:], ones_sb[:], [[1, P]], mybir.AluOpType.is_equal, 0.0,
        base=0, channel_multiplier=-1,
    )
    # selector: sel[k, t*P + m] = 1 iff k == t  (iota val = t - k)
    nc.vector.memset(onesk_sb[:], 1.0)
    nc.gpsimd.affine_select(
        sel_sb[:].rearrange("k (t m) -> k t m", t=TPP),
        onesk_sb[:].rearrange("k (t m) -> k t m", t=TPP),
        [[1, TPP], [0, P]], mybir.AluOpType.is_equal, 0.0,
        base=0, channel_multiplier=-1,
    )

    # ---- loads ----
    nc.sync.dma_start(
        x_sb[:].rearrange("p (t f) -> p t f", t=TPP),
        x.rearrange("(t p) f -> p t f", p=P),
    )
    nc.sync.dma_start(w_sb[:], W.broadcast_to((P, F)))
    nc.sync.dma_start(b_sb[:], b.broadcast_to((P, 1)))

    # ---- scores: s[128t+p] = b + sum_f x[128t+p,f] * W[f] ----
    # (tensor_tensor_reduce would fuse mul+reduce, but it hard-crashes
    #  the device here — NRT_EXEC_UNIT_UNRECOVERABLE — so two ops.)
    nc.vector.tensor_mul(
        tmp_sb[:].rearrange("p (t f) -> p t f", t=TPP),
        x_sb[:].rearrange("p (t f) -> p t f", t=TPP),
        w_sb[:].unsqueeze(1).broadcast_to((P, TPP, F)),
    )
    nc.vector.reduce_sum(
        sc_sb[:], tmp_sb[:].rearrange("p (t f) -> p t f", t=TPP),
        axis=mybir.AxisListType.X,
    )
    nc.vector.tensor_scalar_add(sc_sb[:], sc_sb[:], b_sb[:, 0:1])

    # ---- broadcast scores to all partitions, on-chip (PE) ----
    # transpose: sT[t, p] = sc[p, t] = s[128t+p]
    nc.tensor.matmul(sT_ps[:], lhsT=sc_sb[:], rhs=id_sb[:])
    nc.vector.tensor_copy(s4_sb[:], sT_ps[:])
    # external scores output (off the critical path)
    nc.sync.dma_start(
        scores_out.rearrange("(t p) one -> t (p one)", t=TPP), s4_sb[:]
    )
    # sbc[:, t*P:(t+1)*P] = sel_t.T @ s4 -> every partition gets chunk t
    for t in range(TPP):
        nc.tensor.matmul(
            sbc_ps[:, t * P:(t + 1) * P],
            lhsT=sel_sb[:, t * P:(t + 1) * P],
            rhs=s4_sb[:],
        )
    nc.vector.tensor_copy(sbc_sb[:], sbc_ps[:])

    # ---- mask ----
    nc.vector.max(m8_sb[:], sbc_sb[:])
    # ind = (s == max); mask = ind * (MASK_VAL-1) + 1  (exact in f32 here)
    nc.vector.tensor_scalar(
        mask_sb[:], sbc_sb[:], m8_sb[:, 0:1], None, mybir.AluOpType.is_equal
    )
    nc.vector.tensor_scalar(
        mask_sb[:], mask_sb[:], MASK_VAL - 1.0, 1.0,
        mybir.AluOpType.mult, mybir.AluOpType.add,
    )

    # ---- fill rep_sb: R copies of each row's block slice ----
    # rep_sb[p, (t*R+r)*T + c] = sc[p,t] * mask[c]
    for t in range(TPP):
        out_ap = rep_sb[:, t * R * T:(t + 1) * R * T].rearrange(
            "p (r c) -> p r c", c=T
        )
        in_ap = mask_sb[:].unsqueeze(1).broadcast_to((P, R, T))
        nc.vector.tensor_scalar(
            out_ap, in_ap, sc_sb[:, t:t + 1], None, mybir.AluOpType.mult
        )

    # ---- fan-out DMAs: write each t-slot G times via a step-0 src dim ----
    out_v = rep_out.rearrange("(t p) (g q) -> t p g q", p=P, q=R * T)
    for t in range(TPP):
        src = (
            rep_sb[:, t * R * T:(t + 1) * R * T]
            .unsqueeze(1)
            .broadcast_to((P, G, R * T))
        )
        nc.sync.dma_start(out_v[t], src)


_NC_CACHE = None


def _get_nc():
    global _NC_CACHE
    if _NC_CACHE is None:
        _NC_CACHE = _build()
    return _NC_CACHE


def _run(x, W, b, **run_kwargs):
    nc = _get_nc()
    in_map = {
        "x": np.ascontiguousarray(np.asarray(x, dtype=np.float32)),
        "W": np.ascontiguousarray(np.asarray(W, dtype=np.float32)).reshape(1, F),
        "b": np.ascontiguousarray(np.asarray(b, dtype=np.float32)).reshape(1, 1),
    }
    return run_bass_kernel_spmd(
        nc,
        [dict(in_map) for _ in range(NCORES)],
        core_ids=list(range(NCORES)),
        **run_kwargs,
    )


def kernel(x, W, b):
    res = _run(x, W, b)
    outs = res.results
    full = np.empty((T, 1 + NREP * T), dtype=np.float32)
    full[:, 0:1] = outs[0]["scores"]
    for c in range(NCORES):
        full[:, 1 + c * RPC * T: 1 + (c + 1) * RPC * T] = outs[c]["rep"]
    return full


# revision 12
# speedup vs baseline: 1.6262x; 1.1347x over previous
"""Trainium2 Bass kernel for nn_ExplodedLogit (topk_masking).

Reference computation (x (512,256) f32, W (1,256) f32, b (1,) f32):
    scores = x @ W.T + b                                  (512, 1)
    idx    = argmax(scores)
    mask   = ones(512) with log(1e-46) at idx
    block  = scores * mask[None, :]                       (512, 512)
    out    = concat([scores, tile(block, (1, 512))], 1)   (512, 262145)

Sharding: the 512 identical block repetitions are split across 8
NeuronCores, 64 reps each -> per-core "rep" output (512, 32768) = 64 MB
(memory-bound: this is an HBM-write problem). Every core runs the
identical program: scores/argmax/mask are recomputed redundantly (tiny),
and the per-core slice is materialized with fan-out DMAs that read a
small SBUF block through a step-0 (broadcast) access-pattern dim.

Row layout: r = 128t + p (p = partition 0..127, t = 0..3). The
cross-partition broadcast of the 512 scores (needed to build the mask
along the free dim) runs entirely on-chip: PE transposes scores
[128,4] -> [4,128], then four selector matmuls broadcast each
128-score chunk to all partitions in PSUM — no DRAM round-trip.
"""

import math

import numpy as np

import concourse.bacc as bacc
import concourse.mybir as mybir
import concourse.tile as tile
from concourse.bass_utils import run_bass_kernel_spmd

F32 = mybir.dt.float32
MASK_VAL = float(np.float32(math.log(1e-46)))  # ~ -105.9189

T = 512        # tracks (rows)
F = 256        # features
P = 128        # SBUF partitions
TPP = T // P   # 4 rows per partition (r = 128t + p)
NREP = 512     # total block repetitions in the full output
NCORES = 8
RPC = NREP // NCORES   # 64 reps per core
R = 8                  # reps materialized in SBUF
G = RPC // R           # step-0 groups per fan-out DMA


def _build():
    nc = bacc.Bacc("TRN2", target_bir_lowering=False, debug=False)
    x = nc.dram_tensor("x", [T, F], F32, kind="ExternalInput")
    W = nc.dram_tensor("W", [1, F], F32, kind="ExternalInput")
    b = nc.dram_tensor("b", [1, 1], F32, kind="ExternalInput")
    rep_out = nc.dram_tensor("rep", [T, RPC * T], F32, kind="ExternalOutput")
    scores_out = nc.dram_tensor("scores", [T, 1], F32, kind="ExternalOutput")

    with tile.TileContext(nc) as tc:
        with (
            tc.tile_pool(name="sbuf", bufs=1) as sbuf_pool,
            tc.tile_pool(name="psum", bufs=1, space="PSUM") as psum_pool,
        ):
            _emit(nc, x[:], W[:], b[:], rep_out[:], scores_out[:],
                  sbuf_pool, psum_pool)
    nc.compile()
    return nc


def _emit(nc, x, W, b, rep_out, scores_out, sbuf_pool, psum_pool):
    x_sb = sbuf_pool.tile([P, TPP * F], F32)     # x[128t+p, f] at [p, t*F+f]
    w_sb = sbuf_pool.tile([P, F], F32)
    b_sb = sbuf_pool.tile([P, 1], F32)
    tmp_sb = sbuf_pool.tile([P, TPP * F], F32)
    sc_sb = sbuf_pool.tile([P, TPP], F32)        # scores: s[128t+p] at [p,t]
    ones_sb = sbuf_pool.tile([P, P], F32)
    id_sb = sbuf_pool.tile([P, P], F32)          # 128x128 identity
    onesk_sb = sbuf_pool.tile([TPP, TPP * P], F32)
    sel_sb = sbuf_pool.tile([TPP, TPP * P], F32)  # selector one-hot rows
    s4_sb = sbuf_pool.tile([TPP, P], F32)        # scores, free-dim chunks
    sbc_sb = sbuf_pool.tile([P, T], F32)         # scores bcast: s[c] at [p,c]
    m8_sb = sbuf_pool.tile([P, 8], F32)
    mask_sb = sbuf_pool.tile([P, T], F32)
    rep_sb = sbuf_pool.tile([P, TPP * R * T], F32)

    sT_ps = psum_pool.tile([TPP, P], F32)
    sbc_ps = psum_pool.tile([P, T], F32)

    # ---- constants (overlap with the x load) ----
    nc.vector.memset(ones_sb[:], 1.0)
    # identity: keep ones where (col - row) == 0
    nc.gpsimd.affine_select(
        id_sb[:], ones_sb[:], [[1, P]], mybir.AluOpType.is_equal, 0.0,
        base=0, channel_multiplier=-1,
    )
    # selector: sel[k, t*P + m] = 1 iff k == t  (iota val = t - k)
    nc.vector.memset(onesk_sb[:], 1.0)
    nc.gpsimd.affine_select(
        sel_sb[:].rearrange("k (t m) -> k t m", t=TPP),
        onesk_sb[:].rearrange("k (t m) -> k t m", t=TPP),
        [[1, TPP], [0, P]], mybir.AluOpType.is_equal, 0.0,
        base=0, channel_multiplier=-1,
    )

    # ---- loads ----
    nc.sync.dma_start(
        x_sb[:].rearrange("p (t f) -> p t f", t=TPP),
        x.rearrange("(t p) f -> p t f", p=P),
    )
    nc.sync.dma_start(w_sb[:], W.broadcast_to((P, F)))
    nc.sync.dma_start(b_sb[:], b.broadcast_to((P, 1)))

    # ---- scores: s[128t+p] = b + sum_f x[128t+p,f] * W[f] ----
    # (tensor_tensor_reduce would fuse mul+reduce, but it hard-crashes
    #  the device here — NRT_EXEC_UNIT_UNRECOVERABLE — so two ops.)
    nc.vector.tensor_mul(
        tmp_sb[:].rearrange("p (t f) -> p t f", t=TPP),
        x_sb[:].rearrange("p (t f) -> p t f", t=TPP),
        w_sb[:].unsqueeze(1).broadcast_to((P, TPP, F)),
    )
    nc.vector.reduce_sum(
        sc_sb[:], tmp_sb[:].rearrange("p (t f) -> p t f", t=TPP),
        axis=mybir.AxisListType.X,
    )
    nc.vector.tensor_scalar_add(sc_sb[:], sc_sb[:], b_sb[:, 0:1])

    # ---- broadcast scores to all partitions, on-chip (PE) ----
    # transpose: sT[t, p] = sc[p, t] = s[128t+p]
    nc.tensor.matmul(sT_ps[:], lhsT=sc_sb[:], rhs=id_sb[:])
    nc.vector.tensor_copy(s4_sb[:], sT_ps[:])
    # external scores output (off the critical path)
    nc.sync.dma_start(
        scores_out.rearrange("(t p) one -> t (p one)", t=TPP), s4_sb[:]
    )
    # sbc[:, t*P:(t+1)*P] = sel_t.T @ s4 -> every partition gets chunk t
    for t in range(TPP):
        nc.tensor.matmul(
            sbc_ps[:, t * P:(t + 1) * P],
            lhsT=sel_sb[:, t * P:(t + 1) * P],
            rhs=s4_sb[:],
        )
    nc.vector.tensor_copy(sbc_sb[:], sbc_ps[:])

    # ---- mask ----
    nc.vector.max(m8_sb[:], sbc_sb[:])
    # ind = (s == max); mask = ind * (MASK_VAL-1) + 1  (exact in f32 here)
    nc.vector.tensor_scalar(
        mask_sb[:], sbc_sb[:], m8_sb[:, 0:1], None, mybir.AluOpType.is_equal
    )
    nc.vector.tensor_scalar(
        mask_sb[:], mask_sb[:], MASK_VAL - 1.0, 1.0,
        mybir.AluOpType.mult, mybir.AluOpType.add,
    )

    # ---- fill rep_sb: R copies of each row's block slice ----
    # rep_sb[p, (t*R+r)*T + c] = sc[p,t] * mask[c]
    # t=0 gates the first fan-out DMA: split it across DVE and ACT so the
    # stream starts ~1 us earlier. t=1..3 overlap with streaming anyway.
    h = R // 2
    for t in range(TPP):
        base = t * R * T
        if t == 0:
            nc.vector.tensor_scalar(
                rep_sb[:, base:base + h * T].rearrange("p (r c) -> p r c", c=T),
                mask_sb[:].unsqueeze(1).broadcast_to((P, h, T)),
                sc_sb[:, t:t + 1], None, mybir.AluOpType.mult,
            )
            nc.scalar.activation(
                rep_sb[:, base + h * T:base + R * T].rearrange(
                    "p (r c) -> p r c", c=T
                ),
                mask_sb[:].unsqueeze(1).broadcast_to((P, h, T)),
                mybir.ActivationFunctionType.Copy,
                scale=sc_sb[:, t:t + 1],
            )
        else:
            nc.vector.tensor_scalar(
                rep_sb[:, base:base + R * T].rearrange("p (r c) -> p r c", c=T),
                mask_sb[:].unsqueeze(1).broadcast_to((P, R, T)),
                sc_sb[:, t:t + 1], None, mybir.AluOpType.mult,
            )

    # ---- fan-out DMAs: write each t-slot G times via a step-0 src dim ----
    out_v = rep_out.rearrange("(t p) (g q) -> t p g q", p=P, q=R * T)
    for t in range(TPP):
        src = (
            rep_sb[:, t * R * T:(t + 1) * R * T]
            .unsqueeze(1)
            .broadcast_to((P, G, R * T))
        )
        nc.sync.dma_start(out_v[t], src)


_NC_CACHE = None


def _get_nc():
    global _NC_CACHE
    if _NC_CACHE is None:
        _NC_CACHE = _build()
    return _NC_CACHE


def _run(x, W, b, **run_kwargs):
    nc = _get_nc()
    in_map = {
        "x": np.ascontiguousarray(np.asarray(x, dtype=np.float32)),
        "W": np.ascontiguousarray(np.asarray(W, dtype=np.float32)).reshape(1, F),
        "b": np.ascontiguousarray(np.asarray(b, dtype=np.float32)).reshape(1, 1),
    }
    return run_bass_kernel_spmd(
        nc,
        [dict(in_map) for _ in range(NCORES)],
        core_ids=list(range(NCORES)),
        **run_kwargs,
    )


def kernel(x, W, b):
    res = _run(x, W, b)
    outs = res.results
    full = np.empty((T, 1 + NREP * T), dtype=np.float32)
    full[:, 0:1] = outs[0]["scores"]
    for c in range(NCORES):
        full[:, 1 + c * RPC * T: 1 + (c + 1) * RPC * T] = outs[c]["rep"]
    return full
